# revision 33
# baseline (speedup 1.0000x reference)
"""ChannelAttention (Softmax2d-over-batch) Trainium2 kernel, 8-core SPMD. v2.

Data-parallel over batch (4 samples/core); the only cross-core coupling is
Z[d,c] = sum_b exp(scoresT[b,d,c] - SHIFT), AllReduced in bf16 as two
[5,128,1280] chunks (first kicked at phase-B midpoint, both hidden under the
V-projection phase).

Everything lives in SBUF in 16-bit between phases (no E/V DRAM roundtrips):
  A:  Kt,Qt [hw,c] fp16 from x fp16 (weights fp16, bias via K=1 matmul)
  B:  scoresT psum f32 -> ACT exp -> E bf16 resident; S += E (DVE, bf16)
  V:  V bf16 resident (2-sample-wide matmuls, N=512), overlaps the AllReduce
  R:  r = 1/Z per chunk (DVE fp32 reciprocal, Pool converts bf16<->f32)
  *:  E *= r in place (DVE), so C2 reads E directly as lhsT
  C2: att[c,hw] psum f32 -> fp16 tiles (per 2-sample pair)
  C3: out = (alpha*Wr)@att + alpha*br + x  (alpha folded into Wr,br on host;
      ACT copies psum->fp16, DVE adds resident fp16 x, out written fp16)
"""

import numpy as np

import concourse.bass as bass
import concourse.tile as tile
from concourse import bacc, mybir
from concourse import bass_utils

B, C, S, HW = 32, 1280, 16, 256
P = 128
KC = C // P          # 10 chunks of the channel dim
NCORES = 8
BL = B // NCORES     # 4 samples per core
SHIFT = 45.0
CGROUPS = [(0, 512), (512, 512), (1024, 256)]  # psum-bank-sized column groups
F32 = mybir.dt.float32
F16 = mybir.dt.float16
BF16 = mybir.dt.bfloat16
AF = mybir.ActivationFunctionType
ARC = [(0, 5), (5, 5)]  # AllReduce chunks (dt_ ranges)

_CACHE = {}


def _emit(nc, tc, io):
    ones = io["ones_t"]
    x_d, wk_d, wq_d, wv_d, wr_d = io["x_d"], io["wk_d"], io["wq_d"], io["wv_d"], io["wr_d"]
    s_in, s_out, out_d = io["s_in"], io["s_out"], io["out_d"]

    def bias_row(pool, nm):
        t = pool.tile([1, C], F16, tag=f"row_{nm}", bufs=1, name=f"row_{nm}")
        nc.sync.dma_start(t[:], io[nm].ap())
        return t

    with tc.tile_pool(name="resid", bufs=1) as resid:
        x_sb = resid.tile([P, BL, KC, HW], F16, tag="x")      # 20 KB/p
        e_sb = resid.tile([P, BL, KC, C], BF16, tag="E")      # 100 KB/p
        v_sb = resid.tile([P, BL, KC, HW], BF16, tag="V")     # 20 KB/p
        for b in range(BL):
            nc.sync.dma_start(
                x_sb[:, b], x_d.ap()[b].rearrange("(k p) n -> p k n", p=P)
            )

        # ========= phase A: Kt, Qt resident fp16 =========
        with tc.tile_pool(name="ktqt", bufs=1) as ktqtp:
            kt_sb = ktqtp.tile([P, 2, BL, C], F16, tag="kt")   # 20 KB/p
            qt_sb = ktqtp.tile([P, 2, BL, C], F16, tag="qt")   # 20 KB/p
            with (
                tc.tile_pool(name="wA", bufs=12) as wpA,
                tc.tile_pool(name="psumA", bufs=4, space="PSUM") as psA,
            ):
                brow = {nm: bias_row(wpA, nm) for nm in ("bk", "bq")}
                for wd, bias, dest in ((wk_d, "bk", kt_sb), (wq_d, "bq", qt_sb)):
                    for cgs, cgl in CGROUPS:
                        wt = []
                        for k in range(KC):
                            t = wpA.tile([P, 512], F16, tag="wA")
                            nc.sync.dma_start(
                                t[:, :cgl], wd.ap()[k * P:(k + 1) * P, cgs:cgs + cgl]
                            )
                            wt.append(t)
                        for b in range(BL):
                            for hwt in range(2):
                                ps = psA.tile([P, 512], F32, tag="psA")
                                for k in range(KC):
                                    nc.tensor.matmul(
                                        ps[:, :cgl],
                                        x_sb[:, b, k, hwt * P:(hwt + 1) * P],
                                        wt[k][:, :cgl],
                                        start=(k == 0),
                                        stop=False,
                                    )
                                nc.tensor.matmul(
                                    ps[:, :cgl],
                                    ones[:, :P],
                                    brow[bias][:, cgs:cgs + cgl],
                                    start=False,
                                    stop=True,
                                )
                                nc.vector.tensor_copy(
                                    dest[:, hwt, b, cgs:cgs + cgl], ps[:, :cgl]
                                )

            # ========= phase B: scoresT -> exp -> E (SBUF), S += E =========
            with (
                tc.tile_pool(name="spool", bufs=1) as spool,   # 25 KB/p
                tc.tile_pool(name="psumB", bufs=2, space="PSUM") as psB,
            ):
                s_sb = spool.tile([P, KC, C], BF16, tag="S")
                for dt_ in range(KC):
                    for b in range(BL):
                        ps = psB.tile([P, C], F32, tag="psB")  # 3 banks
                        for cgs, cgl in CGROUPS:
                            for hwt in range(2):
                                nc.tensor.matmul(
                                    ps[:, cgs:cgs + cgl],
                                    qt_sb[:, hwt, b, dt_ * P:(dt_ + 1) * P],
                                    kt_sb[:, hwt, b, cgs:cgs + cgl],
                                    start=(hwt == 0),
                                    stop=(hwt == 1),
                                )
                        esl = e_sb[:, b, dt_]
                        nc.scalar.activation(
                            esl, ps[:], AF.Exp, bias=-SHIFT, scale=1.0
                        )
                        ssl = s_sb[:, dt_]
                        if b == 0:
                            nc.vector.tensor_copy(ssl, esl)
                        else:
                            nc.vector.tensor_add(ssl, ssl, esl)
                    # kick first AllReduce chunk at midpoint
                    if dt_ == ARC[0][0] + ARC[0][1] - 1:
                        for d2 in range(*[ARC[0][0], ARC[0][0] + ARC[0][1]]):
                            nc.scalar.dma_start(s_in.ap()[d2], s_sb[:, d2])
                        _collective(nc, io, ARC[0])
                for d2 in range(ARC[1][0], ARC[1][0] + ARC[1][1]):
                    nc.scalar.dma_start(s_in.ap()[d2], s_sb[:, d2])
            _collective(nc, io, ARC[1])

        # ========= phase V: V bf16 resident (overlaps AllReduce) =========
        # Wv loaded as 10 fat row-block DMAs; psum copies on ACT (idle
        # post-B) so DVE is free for the reciprocal chain below.
        with tc.tile_pool(name="rpool", bufs=1) as rpool:
            r_sb = rpool.tile([P, KC, C], BF16, tag="R")       # 25 KB/p

            # ===== phase V: V bf16 resident (overlaps AllReduce) =====
            # Wv in one fat row-block DMA; psum copies on ACT (idle post-B)
            # so DVE is free for the reciprocal chain below.
            with (
                tc.tile_pool(name="wV", bufs=1) as wpV,
                tc.tile_pool(name="psumV", bufs=3, space="PSUM") as psV,
            ):
                bv_col = wpV.tile([P, KC], F32, tag="bv_col", bufs=1, name="bv_col")
                nc.sync.dma_start(bv_col[:], io["bvc"].ap())
                CH = C // 2
                for half in range(2):
                    wv_t = wpV.tile([P, KC, CH], F16, tag="wV", bufs=2)   # 12.5 KB/p
                    nc.sync.dma_start(
                        wv_t[:],
                        wv_d.ap()[:, half * CH:(half + 1) * CH].rearrange(
                            "(k p) n -> p k n", p=P
                        ),
                    )
                    for vct in range(half * 5, half * 5 + 5):
                        vco = vct * P - half * CH
                        for bp in (0, 2):
                            ps = psV.tile([P, 2, HW], F32, tag="psV")  # 1 bank
                            for ci in range(KC):
                                nc.tensor.matmul(
                                    ps[:], wv_t[:, ci, vco:vco + P],
                                    x_sb[:, bp:bp + 2, ci],
                                    start=(ci == 0), stop=(ci == KC - 1),
                                )
                            nc.scalar.activation(
                                v_sb[:, bp:bp + 2, vct], ps[:], AF.Identity,
                                bias=bv_col[:, vct:vct + 1], scale=1.0,
                            )

                # ---- r = 1/Z per half-chunk (ACT converts, DVE fast recip,
                # ~4e-6 rel, below r's bf16 storage quantization). b0/b1
                # muls interleave per chunk so C2 pair0 can start ASAP.
                with tc.tile_pool(name="zpool", bufs=1) as zp:
                    for dt_ in range(KC):
                        for h in range(2):
                            cs = h * CH
                            zt = zp.tile([P, CH], BF16, tag="zt")
                            nc.sync.dma_start(zt[:], s_out.ap()[dt_, :, cs:cs + CH])
                            zf = zp.tile([P, CH], F32, tag="zf")
                            nc.scalar.copy(zf[:], zt[:])
                            rf = zp.tile([P, CH], F32, tag="rf")
                            nc.vector.reciprocal_approx_fast(rf[:], zf[:])
                            nc.scalar.copy(r_sb[:, dt_, cs:cs + CH], rf[:])
                        for j in range(2):
                            nc.vector.tensor_mul(
                                e_sb[:, j, dt_], e_sb[:, j, dt_], r_sb[:, dt_]
                            )

            # ========= phases C2 + C3 per 2-sample pair =========
            with (
                tc.tile_pool(name="wrot", bufs=2) as wrp,
                tc.tile_pool(name="attp", bufs=KC + 1) as atp,
                tc.tile_pool(name="fin", bufs=2) as finp,
                tc.tile_pool(name="psumC", bufs=4, space="PSUM") as psC,
            ):
                br_col = wrp.tile([P, KC], F32, tag="br_col", bufs=1, name="br_col")
                nc.sync.dma_start(br_col[:], io["brc"].ap())
                for bp in (0, 2):
                    # attn = E * (1/Z), in place on DVE (pair0 done above)
                    if bp:
                        for j in range(2):
                            for dt_ in range(KC):
                                nc.vector.tensor_mul(
                                    e_sb[:, bp + j, dt_], e_sb[:, bp + j, dt_],
                                    r_sb[:, dt_],
                                )
                    att = []   # att tiles [ct][P, 2, HW] fp16
                    for ct in range(KC):
                        t = atp.tile([P, 2, HW], F16, tag="att")
                        for j in range(2):
                            ps = psC.tile([P, HW], F32, tag="psATT")
                            for dt_ in range(KC):
                                nc.tensor.matmul(
                                    ps[:],
                                    e_sb[:, bp + j, dt_, ct * P:(ct + 1) * P],
                                    v_sb[:, bp + j, dt_],
                                    start=(dt_ == 0), stop=(dt_ == KC - 1),
                                )
                            nc.scalar.copy(t[:, j], ps[:])
                        att.append(t)
                    for ot in range(KC):
                        wr_t = wrp.tile([P, KC, P], F16, tag="wrot")  # 2.5 KB/p
                        nc.sync.dma_start(
                            wr_t[:],
                            wr_d.ap()[:, ot * P:(ot + 1) * P].rearrange(
                                "(k p) n -> p k n", p=P
                            ),
                        )
                        ps = psC.tile([P, 2, HW], F32, tag="psREF")  # 1 bank
                        for ct in range(KC):
                            nc.tensor.matmul(
                                ps[:], wr_t[:, ct], att[ct][:],
                                start=(ct == 0), stop=(ct == KC - 1),
                            )
                        tmp = finp.tile([P, 2, HW], F16, tag="tmp")
                        nc.scalar.activation(
                            tmp[:], ps[:], AF.Identity,
                            bias=br_col[:, ot:ot + 1], scale=1.0,
                        )
                        ot_t = finp.tile([P, 2, HW], F16, tag="outT")
                        nc.vector.tensor_add(ot_t[:], tmp[:], x_sb[:, bp:bp + 2, ot])
                        nc.sync.dma_start(
                            out_d.ap()[bp:bp + 2, ot * P:(ot + 1) * P, :].rearrange(
                                "b p n -> p b n"
                            ),
                            ot_t[:],
                        )


def _collective(nc, io, chunk):
    lo, ln = chunk
    if io.get("sim"):
        nc.sync.dma_start(io["s_out"].ap()[lo:lo + ln], io["s_in"].ap()[lo:lo + ln])
    else:
        nc.gpsimd.collective_compute(
            "AllReduce",
            mybir.AluOpType.add,
            replica_groups=[list(range(NCORES))],
            ins=[io["s_in"].ap()[lo:lo + ln]],
            outs=[io["s_out"].ap()[lo:lo + ln]],
        )


def build(alpha: float, dbg: bool = False, nrep: int = 1, sim: bool = False):
    nc = bacc.Bacc(
        "TRN2",
        target_bir_lowering=False,
        debug=False,
        enable_asserts=False,
        num_devices=1 if sim else NCORES,
    )

    io = {"sim": sim}
    io["x_d"] = nc.dram_tensor("x", [BL, C, HW], F16, kind="ExternalInput")
    io["wk_d"] = nc.dram_tensor("wkt", [C, C], F16, kind="ExternalInput")  # Wk.T
    io["wq_d"] = nc.dram_tensor("wqt", [C, C], F16, kind="ExternalInput")
    io["wv_d"] = nc.dram_tensor("wvt", [C, C], F16, kind="ExternalInput")
    io["wr_d"] = nc.dram_tensor("wrt", [C, C], F16, kind="ExternalInput")  # (a*Wr).T
    for nm in ("bk", "bq"):
        io[nm] = nc.dram_tensor(nm, [1, C], F16, kind="ExternalInput")
    # bv / alpha*br in [P, KC] column layout, fed to ACT bias (per partition)
    io["bvc"] = nc.dram_tensor("bvc", [P, KC], F32, kind="ExternalInput")
    io["brc"] = nc.dram_tensor("brc", [P, KC], F32, kind="ExternalInput")
    io["ones_d"] = nc.dram_tensor("ones", [1, 2 * HW], F16, kind="ExternalInput")
    io["out_d"] = nc.dram_tensor("out", [BL, C, HW], F16, kind="ExternalOutput")

    io["s_in"] = nc.dram_tensor("s_in", [KC, P, C], BF16)
    io["s_out"] = nc.dram_tensor("s_out", [KC, P, C], BF16, addr_space="Shared")

    # const AP so ACT Exp can take bias=-SHIFT
    cshift = nc.alloc_sbuf_tensor("const-shift", [128, 1], F32)
    nc.gpsimd.memset(cshift.ap(), -SHIFT)
    nc.const_aps.aps[(F32, -SHIFT)] = cshift.ap()
    nc.all_engine_barrier()

    with tile.TileContext(nc) as tc:
        with tc.tile_pool(name="cpool", bufs=1) as cpool:
            ones = cpool.tile([1, 2 * HW], F16, tag="ones")
            nc.sync.dma_start(ones[:], io["ones_d"].ap())
            io["ones_t"] = ones

            for _ in range(nrep):
                _emit(nc, tc, io)

    nc.compile()
    return nc


def host_inputs(x, Wq, bq, Wk, bk, Wv, bv, Wr, br, alpha):
    """Convert full f32 inputs to the per-core fp16 in_maps."""
    alpha_f = float(np.asarray(alpha).reshape(-1)[0])
    xs = np.ascontiguousarray(
        np.asarray(x, dtype=np.float32).reshape(B, C, HW).astype(np.float16)
    )
    wkt = np.ascontiguousarray(np.asarray(Wk, dtype=np.float32).T.astype(np.float16))
    wqt = np.ascontiguousarray(np.asarray(Wq, dtype=np.float32).T.astype(np.float16))
    wvt = np.ascontiguousarray(np.asarray(Wv, dtype=np.float32).T.astype(np.float16))
    wrt = np.ascontiguousarray(
        (alpha_f * np.asarray(Wr, dtype=np.float32).T).astype(np.float16)
    )
    rows = {
        "bk": np.asarray(bk, dtype=np.float32).reshape(1, C).astype(np.float16),
        "bq": np.asarray(bq, dtype=np.float32).reshape(1, C).astype(np.float16),
        "bvc": np.ascontiguousarray(
            np.asarray(bv, dtype=np.float32).reshape(KC, P).T
        ),
        "brc": np.ascontiguousarray(
            (alpha_f * np.asarray(br, dtype=np.float32)).reshape(KC, P).T
        ),
    }
    in_maps = []
    for c in range(NCORES):
        in_maps.append({
            "x": np.ascontiguousarray(xs[c * BL:(c + 1) * BL]),
            "wkt": wkt, "wqt": wqt, "wvt": wvt, "wrt": wrt,
            "ones": np.ones((1, 2 * HW), dtype=np.float16),
            **rows,
        })
    return in_maps


def kernel(x, Wq, bq, Wk, bk, Wv, bv, Wr, br, alpha):
    alpha_f = float(np.asarray(alpha).reshape(-1)[0])
    key = ("v2",)
    if key not in _CACHE:
        _CACHE[key] = build(alpha_f)
    nc = _CACHE[key]

    in_maps = host_inputs(x, Wq, bq, Wk, bk, Wv, bv, Wr, br, alpha)
    res = bass_utils.run_bass_kernel_spmd(nc, in_maps, core_ids=list(range(NCORES)))
    out = np.concatenate([res.results[c]["out"] for c in range(NCORES)], axis=0)
    return np.ascontiguousarray(out.reshape(B, C, S, S).astype(np.float32))


# revision 36
# speedup vs baseline: 1.1254x; 1.1254x over previous
"""ChannelAttention (Softmax2d-over-batch) Trainium2 kernel, 8-core SPMD. v2.

Data-parallel over batch (4 samples/core); the only cross-core coupling is
Z[d,c] = sum_b exp(scoresT[b,d,c] - SHIFT), AllReduced in bf16 as two
[5,128,1280] chunks (first kicked at phase-B midpoint, both hidden under the
V-projection phase).

Everything lives in SBUF in 16-bit between phases (no E/V DRAM roundtrips):
  A:  Kt,Qt [hw,c] fp16 from x fp16 (weights fp16, bias via K=1 matmul)
  B:  scoresT psum f32 -> ACT exp -> E bf16 resident; S += E (DVE, bf16)
  V:  V bf16 resident (2-sample-wide matmuls, N=512), overlaps the AllReduce
  R:  r = 1/Z per chunk (DVE fp32 reciprocal, Pool converts bf16<->f32)
  *:  E *= r in place (DVE), so C2 reads E directly as lhsT
  C2: att[c,hw] psum f32 -> fp16 tiles (per 2-sample pair)
  C3: out = (alpha*Wr)@att + alpha*br + x  (alpha folded into Wr,br on host;
      ACT copies psum->fp16, DVE adds resident fp16 x, out written fp16)
"""

import numpy as np

import concourse.bass as bass
import concourse.tile as tile
from concourse import bacc, mybir
from concourse import bass_utils

B, C, S, HW = 32, 1280, 16, 256
P = 128
KC = C // P          # 10 chunks of the channel dim
NCORES = 8
BL = B // NCORES     # 4 samples per core
SHIFT = 45.0
CGROUPS = [(0, 512), (512, 512), (1024, 256)]  # psum-bank-sized column groups
F32 = mybir.dt.float32
F16 = mybir.dt.float16
BF16 = mybir.dt.bfloat16
AF = mybir.ActivationFunctionType
ARC = [(0, 5), (5, 5)]  # AllReduce chunks (dt_ ranges)

_CACHE = {}


def _emit(nc, tc, io):
    x_d, wk_d, wq_d, wv_d, wr_d = io["x_d"], io["wk_d"], io["wq_d"], io["wv_d"], io["wr_d"]
    s_in, s_out, out_d = io["s_in"], io["s_out"], io["out_d"]

    def bias_row(pool, nm):
        t = pool.tile([1, C], F16, tag=f"row_{nm}", bufs=1, name=f"row_{nm}")
        nc.sync.dma_start(t[:], io[nm].ap())
        return t

    with tc.tile_pool(name="resid", bufs=1) as resid:
        x_sb = resid.tile([P, BL, KC, HW], F16, tag="x")      # 20 KB/p
        e_sb = resid.tile([P, BL, KC, C], BF16, tag="E")      # 100 KB/p
        v_sb = resid.tile([P, BL, KC, HW], BF16, tag="V")     # 20 KB/p
        for b in range(BL):
            nc.sync.dma_start(
                x_sb[:, b], x_d.ap()[b].rearrange("(k p) n -> p k n", p=P)
            )

        # Wv piece pool opened early so pieces 0/1 prefetch during B and
        # V matmuls start the moment B's last matmul retires.
        with tc.tile_pool(name="wVp", bufs=2) as wpV:
            wv_t = {}

            def wv_piece(vct):
                t = wpV.tile([P, KC, P], F16, tag="wV")        # 2.5 KB/p
                nc.sync.dma_start(
                    t[:],
                    wv_d.ap()[:, vct * P:(vct + 1) * P].rearrange(
                        "(k p) n -> p k n", p=P
                    ),
                )
                wv_t[vct] = t

            # ========= phase A: Kt, Qt resident fp16 =========
            with tc.tile_pool(name="ktqt", bufs=1) as ktqtp:
                kt_sb = ktqtp.tile([P, 2, BL, C], F16, tag="kt")   # 20 KB/p
                qt_sb = ktqtp.tile([P, 2, BL, C], F16, tag="qt")   # 20 KB/p
                with (
                    tc.tile_pool(name="wA", bufs=12) as wpA,
                    tc.tile_pool(name="psumA", bufs=4, space="PSUM") as psA,
                ):
                    ones = wpA.tile([1, P], F16, tag="ones", bufs=1, name="ones")
                    nc.sync.dma_start(ones[:], io["ones_d"].ap()[:, :P])
                    brow = {nm: bias_row(wpA, nm) for nm in ("bk", "bq")}
                    for wd, bias, dest in ((wk_d, "bk", kt_sb), (wq_d, "bq", qt_sb)):
                        for cgs, cgl in CGROUPS:
                            wt = []
                            for k in range(KC):
                                t = wpA.tile([P, 512], F16, tag="wA")
                                nc.sync.dma_start(
                                    t[:, :cgl],
                                    wd.ap()[k * P:(k + 1) * P, cgs:cgs + cgl],
                                )
                                wt.append(t)
                            for b in range(BL):
                                for hwt in range(2):
                                    ps = psA.tile([P, 512], F32, tag="psA")
                                    for k in range(KC):
                                        nc.tensor.matmul(
                                            ps[:, :cgl],
                                            x_sb[:, b, k, hwt * P:(hwt + 1) * P],
                                            wt[k][:, :cgl],
                                            start=(k == 0),
                                            stop=False,
                                        )
                                    nc.tensor.matmul(
                                        ps[:, :cgl],
                                        ones[:, :P],
                                        brow[bias][:, cgs:cgs + cgl],
                                        start=False,
                                        stop=True,
                                    )
                                    nc.vector.tensor_copy(
                                        dest[:, hwt, b, cgs:cgs + cgl], ps[:, :cgl]
                                    )

                # prefetch first two Wv pieces (SP queue, ahead of s_in DMAs)
                wv_piece(0)
                wv_piece(1)

                # ===== phase B: scoresT -> exp -> E (SBUF), S += E =====
                with (
                    tc.tile_pool(name="spool", bufs=3) as spool,   # 7.5 KB/p
                    tc.tile_pool(name="psumB", bufs=2, space="PSUM") as psB,
                ):
                    for dt_ in range(KC):
                        s_t = spool.tile([P, C], BF16, tag="S")
                        for b in range(BL):
                            ps = psB.tile([P, C], F32, tag="psB")  # 3 banks
                            for cgs, cgl in CGROUPS:
                                for hwt in range(2):
                                    nc.tensor.matmul(
                                        ps[:, cgs:cgs + cgl],
                                        qt_sb[:, hwt, b, dt_ * P:(dt_ + 1) * P],
                                        kt_sb[:, hwt, b, cgs:cgs + cgl],
                                        start=(hwt == 0),
                                        stop=(hwt == 1),
                                    )
                            esl = e_sb[:, b, dt_]
                            nc.scalar.activation(
                                esl, ps[:], AF.Exp, bias=-SHIFT, scale=1.0
                            )
                            if b == 0:
                                nc.vector.tensor_copy(s_t[:], esl)
                            else:
                                nc.vector.tensor_add(s_t[:], s_t[:], esl)
                        nc.sync.dma_start(s_in.ap()[dt_], s_t[:])
                        if dt_ == ARC[0][0] + ARC[0][1] - 1:
                            _collective(nc, io, ARC[0])
            _collective(nc, io, ARC[1])

            # ===== phase V: V bf16 resident (overlaps AllReduce) =====
            # Psum copies on ACT (idle post-B) so DVE is free for the
            # reciprocal chain.
            with tc.tile_pool(name="rpool", bufs=1) as rpool:
                r_sb = rpool.tile([P, KC, C], BF16, tag="R")       # 25 KB/p
                with (
                    tc.tile_pool(name="vbias", bufs=1) as vbp,
                    tc.tile_pool(name="psumV", bufs=3, space="PSUM") as psV,
                ):
                    bv_col = vbp.tile([P, KC], F32, tag="bv_col", bufs=1,
                                      name="bv_col")
                    nc.sync.dma_start(bv_col[:], io["bvc"].ap())
                    for vct in range(KC):
                        for bp in (0, 2):
                            ps = psV.tile([P, 2, HW], F32, tag="psV")  # 1 bank
                            for ci in range(KC):
                                nc.tensor.matmul(
                                    ps[:], wv_t[vct][:, ci],
                                    x_sb[:, bp:bp + 2, ci],
                                    start=(ci == 0), stop=(ci == KC - 1),
                                )
                            nc.scalar.activation(
                                v_sb[:, bp:bp + 2, vct], ps[:], AF.Identity,
                                bias=bv_col[:, vct:vct + 1], scale=1.0,
                            )
                        if vct + 2 < KC:
                            wv_piece(vct + 2)

                    # ---- r = 1/Z per half-chunk (ACT converts, DVE fast
                    # recip ~4e-6 rel, below r's bf16 storage quantization).
                    # b0/b1 muls interleave so C2 pair0 can start ASAP.
                    CH = C // 2
                    with tc.tile_pool(name="zpool", bufs=2) as zp:
                        for dt_ in range(KC):
                            for h in range(2):
                                cs = h * CH
                                zt = zp.tile([P, CH], BF16, tag="zt")
                                nc.sync.dma_start(
                                    zt[:], s_out.ap()[dt_, :, cs:cs + CH]
                                )
                                zf = zp.tile([P, CH], F32, tag="zf")
                                nc.scalar.copy(zf[:], zt[:])
                                rf = zp.tile([P, CH], F32, tag="rf")
                                nc.vector.reciprocal_approx_fast(rf[:], zf[:])
                                nc.scalar.copy(r_sb[:, dt_, cs:cs + CH], rf[:])
                            for j in range(2):
                                nc.vector.tensor_mul(
                                    e_sb[:, j, dt_], e_sb[:, j, dt_], r_sb[:, dt_]
                                )

                # ========= phases C2 + C3 per 2-sample pair =========
                with (
                    tc.tile_pool(name="wrot", bufs=2) as wrp,
                    tc.tile_pool(name="attp", bufs=KC + 1) as atp,
                    tc.tile_pool(name="fin", bufs=2) as finp,
                    tc.tile_pool(name="psumC", bufs=4, space="PSUM") as psC,
                ):
                    br_col = wrp.tile([P, KC], F32, tag="br_col", bufs=1,
                                      name="br_col")
                    nc.sync.dma_start(br_col[:], io["brc"].ap())
                    for bp in (0, 2):
                        # attn = E * (1/Z) in place (pair0 done above)
                        if bp:
                            for j in range(2):
                                for dt_ in range(KC):
                                    nc.vector.tensor_mul(
                                        e_sb[:, bp + j, dt_],
                                        e_sb[:, bp + j, dt_],
                                        r_sb[:, dt_],
                                    )
                        att = []   # att tiles [ct][P, 2, HW] fp16
                        for ct in range(KC):
                            t = atp.tile([P, 2, HW], F16, tag="att")
                            for j in range(2):
                                ps = psC.tile([P, HW], F32, tag="psATT")
                                for dt_ in range(KC):
                                    nc.tensor.matmul(
                                        ps[:],
                                        e_sb[:, bp + j, dt_, ct * P:(ct + 1) * P],
                                        v_sb[:, bp + j, dt_],
                                        start=(dt_ == 0), stop=(dt_ == KC - 1),
                                    )
                                nc.scalar.copy(t[:, j], ps[:])
                            att.append(t)
                        for ot in range(KC):
                            wr_t = wrp.tile([P, KC, P], F16, tag="wrot")
                            nc.sync.dma_start(
                                wr_t[:],
                                wr_d.ap()[:, ot * P:(ot + 1) * P].rearrange(
                                    "(k p) n -> p k n", p=P
                                ),
                            )
                            ps = psC.tile([P, 2, HW], F32, tag="psREF")  # 1 bank
                            for ct in range(KC):
                                nc.tensor.matmul(
                                    ps[:], wr_t[:, ct], att[ct][:],
                                    start=(ct == 0), stop=(ct == KC - 1),
                                )
                            tmp = finp.tile([P, 2, HW], F16, tag="tmp")
                            nc.scalar.activation(
                                tmp[:], ps[:], AF.Identity,
                                bias=br_col[:, ot:ot + 1], scale=1.0,
                            )
                            ot_t = finp.tile([P, 2, HW], F16, tag="outT")
                            nc.vector.tensor_add(
                                ot_t[:], tmp[:], x_sb[:, bp:bp + 2, ot]
                            )
                            nc.sync.dma_start(
                                out_d.ap()[bp:bp + 2, ot * P:(ot + 1) * P, :]
                                .rearrange("b p n -> p b n"),
                                ot_t[:],
                            )


def _collective(nc, io, chunk):
    lo, ln = chunk
    if io.get("sim"):
        nc.sync.dma_start(io["s_out"].ap()[lo:lo + ln], io["s_in"].ap()[lo:lo + ln])
    else:
        nc.gpsimd.collective_compute(
            "AllReduce",
            mybir.AluOpType.add,
            replica_groups=[list(range(NCORES))],
            ins=[io["s_in"].ap()[lo:lo + ln]],
            outs=[io["s_out"].ap()[lo:lo + ln]],
        )


def build(alpha: float, dbg: bool = False, nrep: int = 1, sim: bool = False):
    nc = bacc.Bacc(
        "TRN2",
        target_bir_lowering=False,
        debug=False,
        enable_asserts=False,
        num_devices=1 if sim else NCORES,
    )

    io = {"sim": sim}
    io["x_d"] = nc.dram_tensor("x", [BL, C, HW], F16, kind="ExternalInput")
    io["wk_d"] = nc.dram_tensor("wkt", [C, C], F16, kind="ExternalInput")  # Wk.T
    io["wq_d"] = nc.dram_tensor("wqt", [C, C], F16, kind="ExternalInput")
    io["wv_d"] = nc.dram_tensor("wvt", [C, C], F16, kind="ExternalInput")
    io["wr_d"] = nc.dram_tensor("wrt", [C, C], F16, kind="ExternalInput")  # (a*Wr).T
    for nm in ("bk", "bq"):
        io[nm] = nc.dram_tensor(nm, [1, C], F16, kind="ExternalInput")
    # bv / alpha*br in [P, KC] column layout, fed to ACT bias (per partition)
    io["bvc"] = nc.dram_tensor("bvc", [P, KC], F32, kind="ExternalInput")
    io["brc"] = nc.dram_tensor("brc", [P, KC], F32, kind="ExternalInput")
    io["ones_d"] = nc.dram_tensor("ones", [1, 2 * HW], F16, kind="ExternalInput")
    io["out_d"] = nc.dram_tensor("out", [BL, C, HW], F16, kind="ExternalOutput")

    io["s_in"] = nc.dram_tensor("s_in", [KC, P, C], BF16)
    io["s_out"] = nc.dram_tensor("s_out", [KC, P, C], BF16, addr_space="Shared")

    # const AP so ACT Exp can take bias=-SHIFT
    cshift = nc.alloc_sbuf_tensor("const-shift", [128, 1], F32)
    nc.gpsimd.memset(cshift.ap(), -SHIFT)
    nc.const_aps.aps[(F32, -SHIFT)] = cshift.ap()
    nc.all_engine_barrier()

    with tile.TileContext(nc) as tc:
        with tc.tile_pool(name="cpool", bufs=1) as cpool:
            ones = cpool.tile([1, 2 * HW], F16, tag="ones")
            nc.sync.dma_start(ones[:], io["ones_d"].ap())
            io["ones_t"] = ones

            for _ in range(nrep):
                _emit(nc, tc, io)

    nc.compile()
    return nc


def host_inputs(x, Wq, bq, Wk, bk, Wv, bv, Wr, br, alpha):
    """Convert full f32 inputs to the per-core fp16 in_maps."""
    alpha_f = float(np.asarray(alpha).reshape(-1)[0])
    xs = np.ascontiguousarray(
        np.asarray(x, dtype=np.float32).reshape(B, C, HW).astype(np.float16)
    )
    wkt = np.ascontiguousarray(np.asarray(Wk, dtype=np.float32).T.astype(np.float16))
    wqt = np.ascontiguousarray(np.asarray(Wq, dtype=np.float32).T.astype(np.float16))
    wvt = np.ascontiguousarray(np.asarray(Wv, dtype=np.float32).T.astype(np.float16))
    wrt = np.ascontiguousarray(
        (alpha_f * np.asarray(Wr, dtype=np.float32).T).astype(np.float16)
    )
    rows = {
        "bk": np.asarray(bk, dtype=np.float32).reshape(1, C).astype(np.float16),
        "bq": np.asarray(bq, dtype=np.float32).reshape(1, C).astype(np.float16),
        "bvc": np.ascontiguousarray(
            np.asarray(bv, dtype=np.float32).reshape(KC, P).T
        ),
        "brc": np.ascontiguousarray(
            (alpha_f * np.asarray(br, dtype=np.float32)).reshape(KC, P).T
        ),
    }
    in_maps = []
    for c in range(NCORES):
        in_maps.append({
            "x": np.ascontiguousarray(xs[c * BL:(c + 1) * BL]),
            "wkt": wkt, "wqt": wqt, "wvt": wvt, "wrt": wrt,
            "ones": np.ones((1, 2 * HW), dtype=np.float16),
            **rows,
        })
    return in_maps


def kernel(x, Wq, bq, Wk, bk, Wv, bv, Wr, br, alpha):
    alpha_f = float(np.asarray(alpha).reshape(-1)[0])
    key = ("v2",)
    if key not in _CACHE:
        _CACHE[key] = build(alpha_f)
    nc = _CACHE[key]

    in_maps = host_inputs(x, Wq, bq, Wk, bk, Wv, bv, Wr, br, alpha)
    res = bass_utils.run_bass_kernel_spmd(nc, in_maps, core_ids=list(range(NCORES)))
    out = np.concatenate([res.results[c]["out"] for c in range(NCORES)], axis=0)
    return np.ascontiguousarray(out.reshape(B, C, S, S).astype(np.float32))


# revision 37
# speedup vs baseline: 1.3494x; 1.1990x over previous
"""ChannelAttention (Softmax2d-over-batch) Trainium2 kernel, 8-core SPMD. v2.

Data-parallel over batch (4 samples/core); the only cross-core coupling is
Z[d,c] = sum_b exp(scoresT[b,d,c] - SHIFT), AllReduced in bf16 as two
[5,128,1280] chunks (first kicked at phase-B midpoint, both hidden under the
V-projection phase).

Everything lives in SBUF in 16-bit between phases (no E/V DRAM roundtrips):
  A:  Kt,Qt [hw,c] fp16 from x fp16 (weights fp16, bias via K=1 matmul)
  B:  scoresT psum f32 -> ACT exp -> E bf16 resident; S += E (DVE, bf16)
  V:  V bf16 resident (2-sample-wide matmuls, N=512), overlaps the AllReduce
  R:  r = 1/Z per chunk (DVE fp32 reciprocal, Pool converts bf16<->f32)
  *:  E *= r in place (DVE), so C2 reads E directly as lhsT
  C2: att[c,hw] psum f32 -> fp16 tiles (per 2-sample pair)
  C3: out = (alpha*Wr)@att + alpha*br + x  (alpha folded into Wr,br on host;
      ACT copies psum->fp16, DVE adds resident fp16 x, out written fp16)
"""

import numpy as np

import concourse.bass as bass
import concourse.tile as tile
from concourse import bacc, mybir
from concourse import bass_utils

B, C, S, HW = 32, 1280, 16, 256
P = 128
KC = C // P          # 10 chunks of the channel dim
NCORES = 8
BL = B // NCORES     # 4 samples per core
SHIFT = 45.0
CGROUPS = [(0, 512), (512, 512), (1024, 256)]  # psum-bank-sized column groups
F32 = mybir.dt.float32
F16 = mybir.dt.float16
BF16 = mybir.dt.bfloat16
AF = mybir.ActivationFunctionType
ARC = [(0, 5), (5, 5)]  # AllReduce chunks (dt_ ranges)

_CACHE = {}


def _emit(nc, tc, io):
    x_d, wk_d, wq_d, wv_d, wr_d = io["x_d"], io["wk_d"], io["wq_d"], io["wv_d"], io["wr_d"]
    s_in, s_out, out_d = io["s_in"], io["s_out"], io["out_d"]

    def bias_row(pool, nm):
        t = pool.tile([1, C], F16, tag="brow", bufs=1, name=f"row_{nm}")
        nc.sync.dma_start(t[:], io[nm].ap())
        return t

    with tc.tile_pool(name="resid", bufs=1) as resid:
        x_sb = resid.tile([P, BL, KC, HW], F16, tag="x")      # 20 KB/p
        e_sb = resid.tile([P, BL, KC, C], BF16, tag="E")      # 100 KB/p
        v_sb = resid.tile([P, BL, KC, HW], BF16, tag="V")     # 20 KB/p
        for b in range(BL):
            nc.sync.dma_start(
                x_sb[:, b], x_d.ap()[b].rearrange("(k p) n -> p k n", p=P)
            )

        # Wv piece pool opened early so pieces 0/1 prefetch during B and
        # V matmuls start the moment B's last matmul retires.
        with tc.tile_pool(name="wVp", bufs=2) as wpV:
            wv_t = {}

            def wv_piece(vct):
                t = wpV.tile([P, KC, P], F16, tag="wV")        # 2.5 KB/p
                nc.sync.dma_start(
                    t[:],
                    wv_d.ap()[:, vct * P:(vct + 1) * P].rearrange(
                        "(k p) n -> p k n", p=P
                    ),
                )
                wv_t[vct] = t

            # ========= phase A + B interleaved =========
            # Kt fully first; then each Qt column group immediately enables
            # the score chunks (dt) whose qt columns it covers, so ACT's exp
            # stream starts ~1/3 into the Qt pass and AR#1 kicks early.
            with tc.tile_pool(name="ktqt", bufs=1) as ktqtp:
                kt_sb = ktqtp.tile([P, 2, BL, C], F16, tag="kt")   # 20 KB/p
                qt_sb = ktqtp.tile([P, 2, BL, C], F16, tag="qt")   # 20 KB/p
                with (
                    tc.tile_pool(name="wA", bufs=12) as wpA,
                    tc.tile_pool(name="spool", bufs=2) as spool,   # 5 KB/p
                    tc.tile_pool(name="psumA", bufs=2, space="PSUM") as psA,
                    tc.tile_pool(name="psumB", bufs=2, space="PSUM") as psB,
                ):
                    ones = wpA.tile([1, P], F16, tag="ones", bufs=1, name="ones")
                    nc.sync.dma_start(ones[:], io["ones_d"].ap()[:, :P])

                    def proj_cg(wd, brow_t, dest, cgs, cgl):
                        wt = []
                        for k in range(KC):
                            t = wpA.tile([P, 512], F16, tag="wA")
                            nc.sync.dma_start(
                                t[:, :cgl],
                                wd.ap()[k * P:(k + 1) * P, cgs:cgs + cgl],
                            )
                            wt.append(t)
                        for b in range(BL):
                            for hwt in range(2):
                                ps = psA.tile([P, 512], F32, tag="psA")
                                for k in range(KC):
                                    nc.tensor.matmul(
                                        ps[:, :cgl],
                                        x_sb[:, b, k, hwt * P:(hwt + 1) * P],
                                        wt[k][:, :cgl],
                                        start=(k == 0),
                                        stop=False,
                                    )
                                nc.tensor.matmul(
                                    ps[:, :cgl],
                                    ones[:, :P],
                                    brow_t[:, cgs:cgs + cgl],
                                    start=False,
                                    stop=True,
                                )
                                nc.vector.tensor_copy(
                                    dest[:, hwt, b, cgs:cgs + cgl], ps[:, :cgl]
                                )

                    def score_chunk(dt_):
                        s_t = spool.tile([P, C], BF16, tag="S")
                        for b in range(BL):
                            ps = psB.tile([P, C], F32, tag="psB")  # 3 banks
                            for cgs, cgl in CGROUPS:
                                for hwt in range(2):
                                    nc.tensor.matmul(
                                        ps[:, cgs:cgs + cgl],
                                        qt_sb[:, hwt, b, dt_ * P:(dt_ + 1) * P],
                                        kt_sb[:, hwt, b, cgs:cgs + cgl],
                                        start=(hwt == 0),
                                        stop=(hwt == 1),
                                    )
                            esl = e_sb[:, b, dt_]
                            nc.scalar.activation(
                                esl, ps[:], AF.Exp, bias=-SHIFT, scale=1.0
                            )
                            if b == 0:
                                nc.vector.tensor_copy(s_t[:], esl)
                            else:
                                nc.vector.tensor_add(s_t[:], s_t[:], esl)
                        nc.sync.dma_start(s_in.ap()[dt_], s_t[:])
                        if dt_ == ARC[0][0] + ARC[0][1] - 1:
                            _collective(nc, io, ARC[0])

                    brow_t = bias_row(wpA, "bk")
                    for cgs, cgl in CGROUPS:
                        proj_cg(wk_d, brow_t, kt_sb, cgs, cgl)
                    wv_piece(0)
                    wv_piece(1)
                    brow_t = bias_row(wpA, "bq")  # same tag: reuses bk's slot
                    for cgi, (cgs, cgl) in enumerate(CGROUPS):
                        proj_cg(wq_d, brow_t, qt_sb, cgs, cgl)
                        for dt_ in range(cgs // P, (cgs + cgl) // P):
                            score_chunk(dt_)
            _collective(nc, io, ARC[1])

            # ===== phase V: V bf16 resident (overlaps AllReduce) =====
            # Psum copies on ACT (idle post-B) so DVE is free for the
            # reciprocal chain.
            with tc.tile_pool(name="rpool", bufs=1) as rpool:
                r_sb = rpool.tile([P, KC, C], BF16, tag="R")       # 25 KB/p
                with (
                    tc.tile_pool(name="vbias", bufs=1) as vbp,
                    tc.tile_pool(name="psumV", bufs=3, space="PSUM") as psV,
                ):
                    bv_col = vbp.tile([P, KC], F32, tag="bv_col", bufs=1,
                                      name="bv_col")
                    nc.sync.dma_start(bv_col[:], io["bvc"].ap())
                    for vct in range(KC):
                        for bp in (0, 2):
                            ps = psV.tile([P, 2, HW], F32, tag="psV")  # 1 bank
                            for ci in range(KC):
                                nc.tensor.matmul(
                                    ps[:], wv_t[vct][:, ci],
                                    x_sb[:, bp:bp + 2, ci],
                                    start=(ci == 0), stop=(ci == KC - 1),
                                )
                            nc.scalar.activation(
                                v_sb[:, bp:bp + 2, vct], ps[:], AF.Identity,
                                bias=bv_col[:, vct:vct + 1], scale=1.0,
                            )
                        if vct + 2 < KC:
                            wv_piece(vct + 2)

                    # ---- r = 1/Z per half-chunk (ACT converts, DVE fast
                    # recip ~4e-6 rel, below r's bf16 storage quantization).
                    # b0/b1 muls interleave so C2 pair0 can start ASAP.
                    CH = C // 2
                    with tc.tile_pool(name="zpool", bufs=2) as zp:
                        for dt_ in range(KC):
                            for h in range(2):
                                cs = h * CH
                                zt = zp.tile([P, CH], BF16, tag="zt")
                                nc.sync.dma_start(
                                    zt[:], s_out.ap()[dt_, :, cs:cs + CH]
                                )
                                zf = zp.tile([P, CH], F32, tag="zf")
                                nc.scalar.copy(zf[:], zt[:])
                                rf = zp.tile([P, CH], F32, tag="rf")
                                nc.vector.reciprocal_approx_fast(rf[:], zf[:])
                                nc.scalar.copy(r_sb[:, dt_, cs:cs + CH], rf[:])
                            for j in range(2):
                                nc.vector.tensor_mul(
                                    e_sb[:, j, dt_], e_sb[:, j, dt_], r_sb[:, dt_]
                                )

                # ========= phases C2 + C3 per 2-sample pair =========
                with (
                    tc.tile_pool(name="wrot", bufs=2) as wrp,
                    tc.tile_pool(name="attp", bufs=KC + 1) as atp,
                    tc.tile_pool(name="fin", bufs=2) as finp,
                    tc.tile_pool(name="psumC", bufs=4, space="PSUM") as psC,
                ):
                    br_col = wrp.tile([P, KC], F32, tag="br_col", bufs=1,
                                      name="br_col")
                    nc.sync.dma_start(br_col[:], io["brc"].ap())
                    for bp in (0, 2):
                        # attn = E * (1/Z) in place (pair0 done above)
                        if bp:
                            for j in range(2):
                                for dt_ in range(KC):
                                    nc.vector.tensor_mul(
                                        e_sb[:, bp + j, dt_],
                                        e_sb[:, bp + j, dt_],
                                        r_sb[:, dt_],
                                    )
                        att = []   # att tiles [ct][P, 2, HW] fp16
                        for ct in range(KC):
                            t = atp.tile([P, 2, HW], F16, tag="att")
                            for j in range(2):
                                ps = psC.tile([P, HW], F32, tag="psATT")
                                for dt_ in range(KC):
                                    nc.tensor.matmul(
                                        ps[:],
                                        e_sb[:, bp + j, dt_, ct * P:(ct + 1) * P],
                                        v_sb[:, bp + j, dt_],
                                        start=(dt_ == 0), stop=(dt_ == KC - 1),
                                    )
                                nc.scalar.copy(t[:, j], ps[:])
                            att.append(t)
                        for ot in range(KC):
                            wr_t = wrp.tile([P, KC, P], F16, tag="wrot")
                            nc.sync.dma_start(
                                wr_t[:],
                                wr_d.ap()[:, ot * P:(ot + 1) * P].rearrange(
                                    "(k p) n -> p k n", p=P
                                ),
                            )
                            ps = psC.tile([P, 2, HW], F32, tag="psREF")  # 1 bank
                            for ct in range(KC):
                                nc.tensor.matmul(
                                    ps[:], wr_t[:, ct], att[ct][:],
                                    start=(ct == 0), stop=(ct == KC - 1),
                                )
                            tmp = finp.tile([P, 2, HW], F16, tag="tmp")
                            nc.scalar.activation(
                                tmp[:], ps[:], AF.Identity,
                                bias=br_col[:, ot:ot + 1], scale=1.0,
                            )
                            ot_t = finp.tile([P, 2, HW], F16, tag="outT")
                            nc.vector.tensor_add(
                                ot_t[:], tmp[:], x_sb[:, bp:bp + 2, ot]
                            )
                            nc.sync.dma_start(
                                out_d.ap()[bp:bp + 2, ot * P:(ot + 1) * P, :]
                                .rearrange("b p n -> p b n"),
                                ot_t[:],
                            )


def _collective(nc, io, chunk):
    lo, ln = chunk
    if io.get("sim"):
        nc.sync.dma_start(io["s_out"].ap()[lo:lo + ln], io["s_in"].ap()[lo:lo + ln])
    else:
        nc.gpsimd.collective_compute(
            "AllReduce",
            mybir.AluOpType.add,
            replica_groups=[list(range(NCORES))],
            ins=[io["s_in"].ap()[lo:lo + ln]],
            outs=[io["s_out"].ap()[lo:lo + ln]],
        )


def build(alpha: float, dbg: bool = False, nrep: int = 1, sim: bool = False):
    nc = bacc.Bacc(
        "TRN2",
        target_bir_lowering=False,
        debug=False,
        enable_asserts=False,
        num_devices=1 if sim else NCORES,
    )

    io = {"sim": sim}
    io["x_d"] = nc.dram_tensor("x", [BL, C, HW], F16, kind="ExternalInput")
    io["wk_d"] = nc.dram_tensor("wkt", [C, C], F16, kind="ExternalInput")  # Wk.T
    io["wq_d"] = nc.dram_tensor("wqt", [C, C], F16, kind="ExternalInput")
    io["wv_d"] = nc.dram_tensor("wvt", [C, C], F16, kind="ExternalInput")
    io["wr_d"] = nc.dram_tensor("wrt", [C, C], F16, kind="ExternalInput")  # (a*Wr).T
    for nm in ("bk", "bq"):
        io[nm] = nc.dram_tensor(nm, [1, C], F16, kind="ExternalInput")
    # bv / alpha*br in [P, KC] column layout, fed to ACT bias (per partition)
    io["bvc"] = nc.dram_tensor("bvc", [P, KC], F32, kind="ExternalInput")
    io["brc"] = nc.dram_tensor("brc", [P, KC], F32, kind="ExternalInput")
    io["ones_d"] = nc.dram_tensor("ones", [1, 2 * HW], F16, kind="ExternalInput")
    io["out_d"] = nc.dram_tensor("out", [BL, C, HW], F16, kind="ExternalOutput")

    io["s_in"] = nc.dram_tensor("s_in", [KC, P, C], BF16)
    io["s_out"] = nc.dram_tensor("s_out", [KC, P, C], BF16, addr_space="Shared")

    # const AP so ACT Exp can take bias=-SHIFT
    cshift = nc.alloc_sbuf_tensor("const-shift", [128, 1], F32)
    nc.gpsimd.memset(cshift.ap(), -SHIFT)
    nc.const_aps.aps[(F32, -SHIFT)] = cshift.ap()
    nc.all_engine_barrier()

    with tile.TileContext(nc) as tc:
        with tc.tile_pool(name="cpool", bufs=1) as cpool:
            ones = cpool.tile([1, 2 * HW], F16, tag="ones")
            nc.sync.dma_start(ones[:], io["ones_d"].ap())
            io["ones_t"] = ones

            for _ in range(nrep):
                _emit(nc, tc, io)

    nc.compile()
    return nc


def host_inputs(x, Wq, bq, Wk, bk, Wv, bv, Wr, br, alpha):
    """Convert full f32 inputs to the per-core fp16 in_maps."""
    alpha_f = float(np.asarray(alpha).reshape(-1)[0])
    xs = np.ascontiguousarray(
        np.asarray(x, dtype=np.float32).reshape(B, C, HW).astype(np.float16)
    )
    wkt = np.ascontiguousarray(np.asarray(Wk, dtype=np.float32).T.astype(np.float16))
    wqt = np.ascontiguousarray(np.asarray(Wq, dtype=np.float32).T.astype(np.float16))
    wvt = np.ascontiguousarray(np.asarray(Wv, dtype=np.float32).T.astype(np.float16))
    wrt = np.ascontiguousarray(
        (alpha_f * np.asarray(Wr, dtype=np.float32).T).astype(np.float16)
    )
    rows = {
        "bk": np.asarray(bk, dtype=np.float32).reshape(1, C).astype(np.float16),
        "bq": np.asarray(bq, dtype=np.float32).reshape(1, C).astype(np.float16),
        "bvc": np.ascontiguousarray(
            np.asarray(bv, dtype=np.float32).reshape(KC, P).T
        ),
        "brc": np.ascontiguousarray(
            (alpha_f * np.asarray(br, dtype=np.float32)).reshape(KC, P).T
        ),
    }
    in_maps = []
    for c in range(NCORES):
        in_maps.append({
            "x": np.ascontiguousarray(xs[c * BL:(c + 1) * BL]),
            "wkt": wkt, "wqt": wqt, "wvt": wvt, "wrt": wrt,
            "ones": np.ones((1, 2 * HW), dtype=np.float16),
            **rows,
        })
    return in_maps


def kernel(x, Wq, bq, Wk, bk, Wv, bv, Wr, br, alpha):
    alpha_f = float(np.asarray(alpha).reshape(-1)[0])
    key = ("v2",)
    if key not in _CACHE:
        _CACHE[key] = build(alpha_f)
    nc = _CACHE[key]

    in_maps = host_inputs(x, Wq, bq, Wk, bk, Wv, bv, Wr, br, alpha)
    res = bass_utils.run_bass_kernel_spmd(nc, in_maps, core_ids=list(range(NCORES)))
    out = np.concatenate([res.results[c]["out"] for c in range(NCORES)], axis=0)
    return np.ascontiguousarray(out.reshape(B, C, S, S).astype(np.float32))
